# revision 12
# baseline (speedup 1.0000x reference)
"""ASPP + pixel-shuffle upsample + 1x1 project, on 8 TRN2 NeuronCores.

Strategy: data-parallel over batch (B=8 -> 1 image per core). Per core:
  - all convs as matmuls on the PE (bf16 inputs/weights, fp32 PSUM accum)
  - BN folded into conv weights/bias on host
  - 3x3 dilated convs = 9 shifted 1x1 taps accumulated in PSUM; each tap
    computes only its valid (non-zero-padding) region (row/col trimmed APs).
  - x is stored row-major with a 66-col pitch (non-pow2 stride avoids PE
    rhs read bank conflicts) so row-range DMA slices are contiguous: the
    input ships as small slices ordered by first use (sync-engine
    descriptor issue is ~0.6us per dma_start and serial, so issue order
    is the critical path at the start).
  - warmup matmuls on an uninitialized scratch tile start as soon as the
    tensor engine wakes (~6us) and release the HAM clock throttle just as
    the first inputs land.
  - the pixel-shuffle interleave + final f32 conversion happen on the HOST:
    the device writes each branch's projected ReLU output per row-chunk as
    a contiguous bf16 block (halves output HBM traffic, kills strided
    activation writes / descriptor-heavy DMAs). The last 8-row chunk is
    processed as two 4-row half-chunks so the final ACT+DMA tail is short.
"""

import numpy as np
import ml_dtypes

B, CIN, COUT, H = 8, 256, 128, 64
S = 66  # x row pitch (64 cols + 2 pad): non-power-of-two stride
EPS = 1e-5
RATES = (6, 12, 18)
N_CORES = 8
NTAP = 28  # 3 branches * 9 taps + 1 (branch0 1x1)
NWARM = 7

_BF16 = ml_dtypes.bfloat16

# weight block order: per branch, center tap first then (ky,kx) ascending —
# matches in-group emission order so the k=0 groups consume weight blocks
# roughly in DMA arrival order. b1 blocks 0-8, b2 9-17, b3 18-26, b0 27.
_BLK0 = {1: 0, 2: 9, 3: 18}


def _tap_kykx():
    return [(1, 1)] + sorted(
        (ky, kx) for ky in range(3) for kx in range(3) if (ky, kx) != (1, 1)
    )


def _branch_taps(t):
    """[(weight_block, sy, sx)] for branch t, center tap first."""
    if t == 0:
        return [(27, 0, 0)]
    d = RATES[t - 1]
    return [
        (_BLK0[t] + i, (ky - 1) * d, (kx - 1) * d)
        for i, (ky, kx) in enumerate(_tap_kykx())
    ]


# (k, row0, nrows) chunk list: seven 8-row chunks + two 4-row half-chunks
_CHUNKS = [(k, 8 * k, 8) for k in range(7)] + [(7, 56, 4), (8, 60, 4)]


def build_program():
    import concourse.mybir as mybir
    import concourse.tile as tile
    from concourse import bacc

    f32, bf16 = mybir.dt.float32, mybir.dt.bfloat16
    Relu = mybir.ActivationFunctionType.Relu

    nc = bacc.Bacc("TRN2", target_bir_lowering=False, debug=False)
    xp = nc.dram_tensor("xp", [2, 128, H * S], bf16, kind="ExternalInput")
    wb = nc.dram_tensor("wb", [2, 128, NTAP * 128], bf16, kind="ExternalInput")
    wp = nc.dram_tensor("wp", [128, 128], bf16, kind="ExternalInput")
    bias = nc.dram_tensor("bias", [128, 5], f32, kind="ExternalInput")
    # branch t's projected rows 8k..8k+8 at block (4k+t)*512 (row-major
    # [a, c]); the k=7 half-chunks write the two halves of block (28+t)
    out = nc.dram_tensor("out", [128, 32 * 512], bf16, kind="ExternalOutput")

    with tile.TileContext(nc) as tc:
        with (
            tc.tile_pool(name="const", bufs=1) as cpool,
            tc.tile_pool(name="bf", bufs=3) as bfpool,
            tc.tile_pool(name="ob", bufs=6) as obpool,
            tc.tile_pool(name="psA", bufs=4, space="PSUM") as psA,
            tc.tile_pool(name="psB", bufs=3, space="PSUM") as psB,
        ):
            # PE warm-up: all engines barrier at ~7.5us (framework preamble),
            # so the gpsimd memset + warmup matmuls start ~7.8us; NWARM sized
            # so warmup drains right as the first input DMAs land and the
            # first real matmuls continue warming the HAM clock throttle
            scratch = cpool.tile([128, 512], bf16, tag="scratch")
            nc.gpsimd.memset(scratch[:], 0.0)
            psW = psA.tile([128, 512], f32, tag="warm", bufs=1)
            for i in range(NWARM):
                nc.tensor.matmul(
                    psW[:], lhsT=scratch[:, :128], rhs=scratch[:],
                    start=(i == 0), stop=(i == NWARM - 1), skip_group_check=True,
                )

            bt = cpool.tile([128, 5], f32, tag="bias")
            wpt = cpool.tile([128, 128], bf16, tag="wp")
            wt = [
                cpool.tile([128, NTAP * 128], bf16, tag=f"w{c}", name=f"w{c}")
                for c in range(2)
            ]
            xtile = [
                cpool.tile([128, H * S], bf16, tag=f"x{c}", name=f"x{c}")
                for c in range(2)
            ]
            x3 = [xtile[c].rearrange("p (h w) -> p h w", w=S) for c in range(2)]

            def dx(eng, c, r0, r1):
                eng.dma_start(
                    out=xtile[c][:, r0 * S : r1 * S], in_=xp[c][:, r0 * S : r1 * S]
                )

            def dw(eng, c, b0_, b1_):
                eng.dma_start(
                    out=wt[c][:, b0_ * 128 : b1_ * 128],
                    in_=wb[c][:, b0_ * 128 : b1_ * 128],
                )

            # input DMA issue: descriptor generation costs ~0.6us per
            # dma_start ON THE ISSUING engine, serially — spread the wave
            # round-robin across the three DMA-capable engines (sync/SP,
            # scalar/Activation, gpsimd), in first-use order per engine
            SY, SC, GP = nc.sync, nc.scalar, nc.gpsimd
            dw(SY, 0, 0, 9)        # b1 cin0 taps
            dx(SC, 0, 0, 8)        # b1/k0 center+sy0 taps read rows 0..7
            dx(GP, 0, 8, 16)       # sy=+6 taps read up to row 13
            dw(SY, 1, 0, 9)
            dx(SC, 1, 0, 16)
            GP.dma_start(out=bt, in_=bias[:])
            SY.dma_start(out=wpt, in_=wp[:])
            dw(SC, 0, 9, 18)       # b2
            dx(GP, 0, 16, 32)
            dw(SY, 1, 9, 18)
            dx(SC, 1, 16, 32)
            dw(GP, 0, 18, 27)      # b3
            dw(SY, 1, 18, 27)
            dw(SC, 0, 27, 28)      # b0
            dw(GP, 1, 27, 28)
            dx(SY, 0, 32, 64)
            dx(SC, 1, 32, 64)

            def emit_group(ps, row0, nr, t):
                """Conv matmuls of branch t, output rows row0..row0+nr, both
                cin chunks, accumulating into ps[:, :nr*64] ([h=nr, w=64])."""
                mms = []
                for c in (0, 1):
                    for blk, sy, sx in _branch_taps(t):
                        if row0 + nr + sy <= 0 or row0 + sy >= H:
                            continue  # every row reads zero padding
                        a0 = max(0, -sy - row0)
                        a1 = min(nr, H - sy - row0)
                        c0, c1 = max(0, -sx), min(H, H - sx)
                        mms.append((blk, sy, sx, a0, a1, c0, c1, c))
                n = len(mms)
                ps3 = ps.rearrange("p (h w) -> p h w", w=H)
                for i, (blk, sy, sx, a0, a1, c0, c1, c) in enumerate(mms):
                    r0 = row0 + sy + a0
                    rhs = x3[c][:, r0 : r0 + (a1 - a0), c0 + sx : c1 + sx]
                    if (c0, c1) == (0, H):
                        dst = ps[:, a0 * H : a1 * H]
                    else:
                        dst = ps3[:, a0:a1, c0:c1]
                    nc.tensor.matmul(
                        dst,
                        lhsT=wt[c][:, blk * 128 : (blk + 1) * 128],
                        rhs=rhs,
                        start=(i == 0),
                        stop=(i == n - 1),
                    )

            for k, row0, nr in _CHUNKS:
                # early chunks: order branches so each group's weights/x rows
                # have landed by the time the PE reaches it
                order = [1, 2, 3, 0] if k == 0 else ([1, 2, 0, 3] if k == 1 else range(4))
                for t in order:
                    # PSUM tiles stay full [128,512] (bank-aligned); the 4-row
                    # half-chunks use only the first 256 columns
                    nw = nr * H
                    ps = psA.tile([128, 512], f32, tag="ps")
                    emit_group(ps, row0, nr, t)
                    bftile = bfpool.tile([128, nw], bf16, tag="bf")
                    nc.scalar.activation(bftile[:], ps[:, :nw], Relu, bias=bt[:, t : t + 1])
                    ps2 = psB.tile([128, 512], f32, tag="ps2")
                    nc.tensor.matmul(
                        ps2[:, :nw], lhsT=wpt[:], rhs=bftile[:], start=True, stop=True
                    )
                    ob = obpool.tile([128, nw], bf16, tag="ob")
                    # post-proj ReLU+bias on the (idle) vector engine: keeps
                    # the scalar queue short so proj matmuls never wait, and
                    # shortens the final-chunk tail
                    nc.vector.tensor_scalar(
                        ob[:], ps2[:, :nw], bt[:, 4:5], 0.0,
                        mybir.AluOpType.add, mybir.AluOpType.max,
                    )
                    blk = (4 * min(k, 7) + t) * 512 + (row0 - 8 * min(k, 7)) * H
                    nc.sync.dma_start(out=out[:, blk : blk + nw], in_=ob[:])
    nc.compile()
    return nc


def host_prep_weights(inputs):
    f32 = np.float32
    scales, biases = [], []
    for t in ("0", "1", "2", "3", "p"):
        g = np.asarray(inputs[f"g{t}"], f32)
        b = np.asarray(inputs[f"b{t}"], f32)
        m = np.asarray(inputs[f"m{t}"], f32)
        v = np.asarray(inputs[f"v{t}"], f32)
        s = g / np.sqrt(v + EPS)
        scales.append(s)
        biases.append((b - m * s).astype(f32))
    bias_arr = np.stack(biases, axis=1).astype(f32)  # (128, 5)

    wtaps = np.zeros((NTAP, CIN, COUT), f32)  # [blk, ci, co]
    order = _tap_kykx()
    for bi, key in enumerate(("w1", "w2", "w3")):
        w = np.asarray(inputs[key], f32) * scales[bi + 1][:, None, None, None]
        for i, (ky, kx) in enumerate(order):
            wtaps[_BLK0[bi + 1] + i] = w[:, :, ky, kx].T
    w0 = np.asarray(inputs["w0"], f32)[:, :, 0, 0] * scales[0][:, None]  # (co, ci)
    wtaps[27] = w0.T
    wb = (
        wtaps.reshape(NTAP, 2, 128, COUT)
        .transpose(1, 2, 0, 3)
        .reshape(2, 128, NTAP * COUT)
        .astype(_BF16)
    )
    wpT = (
        (np.asarray(inputs["wp"], f32)[:, :, 0, 0] * scales[4][:, None])
        .T.astype(_BF16)
        .copy()
    )
    return wb, wpT, bias_arr


def host_prep_x(x):
    # row-major with 66-col pitch; no transpose needed
    x = np.asarray(x, np.float32).reshape(B, 2, 128, H, H)
    xt = np.zeros((B, 2, 128, H, S), np.float32)
    xt[:, :, :, :, :H] = x
    return xt.reshape(B, 2, 128, H * S).astype(_BF16)


def make_in_maps(inputs):
    wb, wpT, bias_arr = host_prep_weights(inputs)
    xq = host_prep_x(inputs["x"])
    return [{"xp": xq[b], "wb": wb, "wp": wpT, "bias": bias_arr} for b in range(B)]


def host_interleave(raw):
    """Device out [128, 32*512] bf16 -> (COUT, 2H, 2H) f32.

    Block (4k+t) holds branch t's projected rows 8k..8k+8 (row-major
    [a=8, c=64]); t = 2*r + cc selects output row/col parity.
    """
    arr = np.asarray(raw, np.float32).reshape(COUT, 8, 2, 2, 8, H)
    return arr.transpose(0, 1, 4, 2, 5, 3).reshape(COUT, 2 * H, 2 * H)


_NC_CACHE = []


def kernel(**inputs):
    from concourse import bass_utils

    if not _NC_CACHE:
        _NC_CACHE.append(build_program())
    nc = _NC_CACHE[0]
    in_maps = make_in_maps(inputs)
    res = bass_utils.run_bass_kernel_spmd(nc, in_maps, core_ids=list(range(N_CORES)))
    return np.stack([host_interleave(r["out"]) for r in res.results]).astype(np.float32)


# revision 14
# speedup vs baseline: 1.0273x; 1.0273x over previous
"""ASPP + pixel-shuffle upsample + 1x1 project, on 8 TRN2 NeuronCores.

Strategy: data-parallel over batch (B=8 -> 1 image per core). Per core:
  - all convs as matmuls on the PE (bf16 inputs/weights, fp32 PSUM accum)
  - BN folded into conv weights/bias on host
  - 3x3 dilated convs = 9 shifted 1x1 taps accumulated in PSUM; each tap
    computes only its valid (non-zero-padding) region (row/col trimmed APs).
  - x is stored row-major with a 66-col pitch (non-pow2 stride avoids PE
    rhs read bank conflicts) so row-range DMA slices are contiguous: the
    input ships as small slices ordered by first use (sync-engine
    descriptor issue is ~0.6us per dma_start and serial, so issue order
    is the critical path at the start).
  - warmup matmuls on an uninitialized scratch tile start as soon as the
    tensor engine wakes (~6us) and release the HAM clock throttle just as
    the first inputs land.
  - the pixel-shuffle interleave + final f32 conversion happen on the HOST:
    the device writes each branch's projected ReLU output per row-chunk as
    a contiguous bf16 block (halves output HBM traffic, kills strided
    activation writes / descriptor-heavy DMAs). The last 8-row chunk is
    processed as two 4-row half-chunks so the final ACT+DMA tail is short.
"""

import numpy as np
import ml_dtypes

B, CIN, COUT, H = 8, 256, 128, 64
S = 66  # x row pitch (64 cols + 2 pad): non-power-of-two stride
EPS = 1e-5
RATES = (6, 12, 18)
N_CORES = 8
NTAP = 28  # 3 branches * 9 taps + 1 (branch0 1x1)
NWARM = 7

_BF16 = ml_dtypes.bfloat16

# weight block order: per branch, center tap first then (ky,kx) ascending —
# matches in-group emission order so the k=0 groups consume weight blocks
# roughly in DMA arrival order. b1 blocks 0-8, b2 9-17, b3 18-26, b0 27.
_BLK0 = {1: 0, 2: 9, 3: 18}


def _tap_kykx():
    return [(1, 1)] + sorted(
        (ky, kx) for ky in range(3) for kx in range(3) if (ky, kx) != (1, 1)
    )


def _branch_taps(t):
    """[(weight_block, sy, sx)] for branch t, center tap first."""
    if t == 0:
        return [(27, 0, 0)]
    d = RATES[t - 1]
    return [
        (_BLK0[t] + i, (ky - 1) * d, (kx - 1) * d)
        for i, (ky, kx) in enumerate(_tap_kykx())
    ]


# (k, row0, nrows) chunk list: seven 8-row chunks + two 4-row half-chunks
_CHUNKS = [(k, 8 * k, 8) for k in range(7)] + [(7, 56, 4), (8, 60, 4)]


def build_program():
    import concourse.mybir as mybir
    import concourse.tile as tile
    from concourse import bacc

    f32, bf16 = mybir.dt.float32, mybir.dt.bfloat16
    Relu = mybir.ActivationFunctionType.Relu

    nc = bacc.Bacc("TRN2", target_bir_lowering=False, debug=False)
    xp = nc.dram_tensor("xp", [2, 128, H * S], bf16, kind="ExternalInput")
    wb = nc.dram_tensor("wb", [2, 128, NTAP * 128], bf16, kind="ExternalInput")
    wp = nc.dram_tensor("wp", [128, 128], bf16, kind="ExternalInput")
    bias = nc.dram_tensor("bias", [128, 5], f32, kind="ExternalInput")
    # branch t's projected rows 8k..8k+8 at block (4k+t)*512 (row-major
    # [a, c]); the k=7 half-chunks write the two halves of block (28+t)
    out = nc.dram_tensor("out", [128, 32 * 512], bf16, kind="ExternalOutput")

    with tile.TileContext(nc) as tc:
        with (
            tc.tile_pool(name="const", bufs=1) as cpool,
            tc.tile_pool(name="bf", bufs=3) as bfpool,
            tc.tile_pool(name="ob", bufs=6) as obpool,
            tc.tile_pool(name="psA", bufs=4, space="PSUM") as psA,
            tc.tile_pool(name="psB", bufs=3, space="PSUM") as psB,
        ):
            # PE warm-up: all engines barrier at ~7.5us (framework preamble),
            # so the gpsimd memset + warmup matmuls start ~7.8us; NWARM sized
            # so warmup drains right as the first input DMAs land and the
            # first real matmuls continue warming the HAM clock throttle
            scratch = cpool.tile([128, 512], bf16, tag="scratch")
            nc.gpsimd.memset(scratch[:], 0.0)
            psW = psA.tile([128, 512], f32, tag="warm", bufs=1)
            for i in range(NWARM):
                nc.tensor.matmul(
                    psW[:], lhsT=scratch[:, :128], rhs=scratch[:],
                    start=(i == 0), stop=(i == NWARM - 1), skip_group_check=True,
                )

            bt = cpool.tile([128, 5], f32, tag="bias")
            wpt = cpool.tile([128, 128], bf16, tag="wp")
            wt = [
                cpool.tile([128, NTAP * 128], bf16, tag=f"w{c}", name=f"w{c}")
                for c in range(2)
            ]
            xtile = [
                cpool.tile([128, H * S], bf16, tag=f"x{c}", name=f"x{c}")
                for c in range(2)
            ]
            x3 = [xtile[c].rearrange("p (h w) -> p h w", w=S) for c in range(2)]

            def dx(eng, c, r0, r1):
                eng.dma_start(
                    out=xtile[c][:, r0 * S : r1 * S], in_=xp[c][:, r0 * S : r1 * S]
                )

            def dw(eng, c, b0_, b1_):
                eng.dma_start(
                    out=wt[c][:, b0_ * 128 : b1_ * 128],
                    in_=wb[c][:, b0_ * 128 : b1_ * 128],
                )

            # input DMA issue: all on sync, serial, in first-use order —
            # serial issue makes delivery order match need order at full
            # per-chain bandwidth (parallel multi-engine issue measured WORSE:
            # concurrent chains dilute bandwidth away from the critical head)
            SY = nc.sync
            dw(SY, 0, 0, 9)        # b1 cin0 taps
            dx(SY, 0, 0, 8)        # b1/k0 center+sy0 taps read rows 0..7
            dx(SY, 0, 8, 16)       # sy=+6 taps read up to row 13
            dw(SY, 1, 0, 9)
            dx(SY, 1, 0, 16)
            SY.dma_start(out=bt, in_=bias[:])
            SY.dma_start(out=wpt, in_=wp[:])
            dx(SY, 0, 16, 32)      # k1/b1 sy=+6 taps read up to row 21
            dw(SY, 0, 9, 18)       # b2
            dx(SY, 1, 16, 32)
            dw(SY, 1, 9, 18)
            dw(SY, 0, 18, 27)      # b3
            dw(SY, 1, 18, 27)
            dw(SY, 0, 27, 28)      # b0
            dw(SY, 1, 27, 28)
            dx(SY, 0, 32, 64)
            dx(SY, 1, 32, 64)

            def emit_group(ps, row0, nr, t):
                """Conv matmuls of branch t, output rows row0..row0+nr, both
                cin chunks, accumulating into ps[:, :nr*64] ([h=nr, w=64])."""
                mms = []
                for c in (0, 1):
                    for blk, sy, sx in _branch_taps(t):
                        if row0 + nr + sy <= 0 or row0 + sy >= H:
                            continue  # every row reads zero padding
                        a0 = max(0, -sy - row0)
                        a1 = min(nr, H - sy - row0)
                        c0, c1 = max(0, -sx), min(H, H - sx)
                        mms.append((blk, sy, sx, a0, a1, c0, c1, c))
                n = len(mms)
                ps3 = ps.rearrange("p (h w) -> p h w", w=H)
                for i, (blk, sy, sx, a0, a1, c0, c1, c) in enumerate(mms):
                    r0 = row0 + sy + a0
                    rhs = x3[c][:, r0 : r0 + (a1 - a0), c0 + sx : c1 + sx]
                    if (c0, c1) == (0, H):
                        dst = ps[:, a0 * H : a1 * H]
                    else:
                        dst = ps3[:, a0:a1, c0:c1]
                    nc.tensor.matmul(
                        dst,
                        lhsT=wt[c][:, blk * 128 : (blk + 1) * 128],
                        rhs=rhs,
                        start=(i == 0),
                        stop=(i == n - 1),
                    )

            # k0 and k1 interleaved per branch: each branch's weight transfer
            # (0.6MB) feeds TWO chunks of compute back to back, halving the
            # early input-demand rate to below the DMA supply rate
            sched = []
            for t in (1, 2, 3, 0):
                sched.append((_CHUNKS[0], t))
                sched.append((_CHUNKS[1], t))
            for ch in _CHUNKS[2:]:
                for t in range(4):
                    sched.append((ch, t))

            for (k, row0, nr), t in sched:
                if True:
                    # PSUM tiles stay full [128,512] (bank-aligned); the 4-row
                    # half-chunks use only the first 256 columns
                    nw = nr * H
                    ps = psA.tile([128, 512], f32, tag="ps")
                    emit_group(ps, row0, nr, t)
                    bftile = bfpool.tile([128, nw], bf16, tag="bf")
                    nc.scalar.activation(bftile[:], ps[:, :nw], Relu, bias=bt[:, t : t + 1])
                    ps2 = psB.tile([128, 512], f32, tag="ps2")
                    nc.tensor.matmul(
                        ps2[:, :nw], lhsT=wpt[:], rhs=bftile[:], start=True, stop=True
                    )
                    ob = obpool.tile([128, nw], bf16, tag="ob")
                    # post-proj ReLU+bias on the (idle) vector engine: keeps
                    # the scalar queue short so proj matmuls never wait, and
                    # shortens the final-chunk tail
                    nc.vector.tensor_scalar(
                        ob[:], ps2[:, :nw], bt[:, 4:5], 0.0,
                        mybir.AluOpType.add, mybir.AluOpType.max,
                    )
                    blk = (4 * min(k, 7) + t) * 512 + (row0 - 8 * min(k, 7)) * H
                    nc.sync.dma_start(out=out[:, blk : blk + nw], in_=ob[:])
    nc.compile()
    return nc


def host_prep_weights(inputs):
    f32 = np.float32
    scales, biases = [], []
    for t in ("0", "1", "2", "3", "p"):
        g = np.asarray(inputs[f"g{t}"], f32)
        b = np.asarray(inputs[f"b{t}"], f32)
        m = np.asarray(inputs[f"m{t}"], f32)
        v = np.asarray(inputs[f"v{t}"], f32)
        s = g / np.sqrt(v + EPS)
        scales.append(s)
        biases.append((b - m * s).astype(f32))
    bias_arr = np.stack(biases, axis=1).astype(f32)  # (128, 5)

    wtaps = np.zeros((NTAP, CIN, COUT), f32)  # [blk, ci, co]
    order = _tap_kykx()
    for bi, key in enumerate(("w1", "w2", "w3")):
        w = np.asarray(inputs[key], f32) * scales[bi + 1][:, None, None, None]
        for i, (ky, kx) in enumerate(order):
            wtaps[_BLK0[bi + 1] + i] = w[:, :, ky, kx].T
    w0 = np.asarray(inputs["w0"], f32)[:, :, 0, 0] * scales[0][:, None]  # (co, ci)
    wtaps[27] = w0.T
    wb = (
        wtaps.reshape(NTAP, 2, 128, COUT)
        .transpose(1, 2, 0, 3)
        .reshape(2, 128, NTAP * COUT)
        .astype(_BF16)
    )
    wpT = (
        (np.asarray(inputs["wp"], f32)[:, :, 0, 0] * scales[4][:, None])
        .T.astype(_BF16)
        .copy()
    )
    return wb, wpT, bias_arr


def host_prep_x(x):
    # row-major with 66-col pitch; no transpose needed
    x = np.asarray(x, np.float32).reshape(B, 2, 128, H, H)
    xt = np.zeros((B, 2, 128, H, S), np.float32)
    xt[:, :, :, :, :H] = x
    return xt.reshape(B, 2, 128, H * S).astype(_BF16)


def make_in_maps(inputs):
    wb, wpT, bias_arr = host_prep_weights(inputs)
    xq = host_prep_x(inputs["x"])
    return [{"xp": xq[b], "wb": wb, "wp": wpT, "bias": bias_arr} for b in range(B)]


def host_interleave(raw):
    """Device out [128, 32*512] bf16 -> (COUT, 2H, 2H) f32.

    Block (4k+t) holds branch t's projected rows 8k..8k+8 (row-major
    [a=8, c=64]); t = 2*r + cc selects output row/col parity.
    """
    arr = np.asarray(raw, np.float32).reshape(COUT, 8, 2, 2, 8, H)
    return arr.transpose(0, 1, 4, 2, 5, 3).reshape(COUT, 2 * H, 2 * H)


_NC_CACHE = []


def kernel(**inputs):
    from concourse import bass_utils

    if not _NC_CACHE:
        _NC_CACHE.append(build_program())
    nc = _NC_CACHE[0]
    in_maps = make_in_maps(inputs)
    res = bass_utils.run_bass_kernel_spmd(nc, in_maps, core_ids=list(range(N_CORES)))
    return np.stack([host_interleave(r["out"]) for r in res.results]).astype(np.float32)
